# revision 5
# baseline (speedup 1.0000x reference)
"""Trainium2 Bass kernel for CapsuleLayer (nn_CapsuleLayer_45552423142009).

Two-pass structure:
  Pass 1 (dense PE stream, keeps HAM at K=8/8): per tile, DMA x,
    PE-transpose, mm1, squash -> u tile parked in SBUF (32 x 1KB/part).
  Pass 2 (routing): logits live in PSUM the whole time -- agreement
    accumulates b via start=False matmuls, and the softmax log-sum-exp
    subtraction is a negated-selector matmul into the same bank
    (softmax is shift-invariant, so the residual per-p shift cancels).
  v = s*g is computed every iteration and agreements contract v
    directly (u_hat never materialized).
  ACT uses only {Ln, Exp, Square, Identity, Copy}: one table set.
"""

import sys
import numpy as np

sys.path.insert(0, "/opt/trn_rl_repo")

from concourse import bass, bacc, mybir  # noqa: E402
from concourse import tile  # noqa: E402
from concourse.bass_utils import run_bass_kernel_spmd  # noqa: E402
from concourse.alu_op_type import AluOpType  # noqa: E402

# Pin ACT to the one table set covering every function used here
# (Ln, Exp, Square, Identity, Copy); placement otherwise first-fits
# Ln->natural_log / Exp->exp_and_others and thrashes table loads.
from concourse import hw_specs as _hw_specs  # noqa: E402

_ORIG_GAT = _hw_specs.get_activation_tables
_KEEP_SET = "natural_log_exp_and_others"


def _pinned_tables(arch):
    tabs = _ORIG_GAT(arch)
    return {k: (v if k == _KEEP_SET else set()) for k, v in tabs.items()}


_hw_specs.get_activation_tables = _pinned_tables
bacc.get_activation_tables = _pinned_tables

F32 = mybir.dt.float32
F32R = mybir.dt.float32r
BF16 = mybir.dt.bfloat16
AF = mybir.ActivationFunctionType

B = 131072
D = 768
P = 8
PD = 16
C = 5
CD = 16
NCORES = 8
BC = B // NCORES          # 16384 batch rows per core
NB = 512                  # batch columns per tile
NT = BC // NB             # 32 tiles

# bf16 const blob column offsets
OFF_WP = 0                # [128, 768]  mm1 weights, 6 chunks of [128,128]
OFF_SSEL = 768            # [128, 8]    sum 16-groups -> p
OFF_SBC = 776             # [8, 128]    broadcast p -> (p,i)
OFF_M0 = 904              # [128, 80]   0.2*W flat: s0 = M0^T u
OFF_JSEL = 984            # [80, 5]     sum j at fixed c
OFF_G80 = 1029            # [5, 80]     broadcast c -> (c,j)
OFF_CSEL = 1109           # [80, 8]     sum c at fixed p (logits at c*16+p)
OFF_CBCN = 1117           # [8, 80]     NEGATED broadcast p -> (c*16+p)
OFF_WTA = 1197            # [80, 640]   5 x [16,128] blocks: wv_c = WTa_c^T v
OFF_ASEL = 1837           # [128, 400]  5 x [128,80]: sum i -> (c*16+p)
OFF_BSEL = 2237           # [80, 640]   5 x [8,128] blocks: bcast (c*16+p)->(p,i)
OFF_WF = 2877             # [128, 400]  5 x [128,80]: s_c = Wf_c^T t_c
CB_W = OFF_WF + 400
CF_W = 129                # f32 blob: ident(128) + bp(1)


def build_consts(Wp, bp, W):
    """Return (cstf [128,129] f32, cstb [128,CB_W] bf16)."""
    Wp = np.asarray(Wp, np.float32)
    bp = np.asarray(bp, np.float32)
    W = np.asarray(W, np.float32)

    cstf = np.zeros((128, CF_W), np.float32)
    cstf[:, 0:128] = np.eye(128, dtype=np.float32)
    cstf[:, 128] = bp.reshape(128)

    cb = np.zeros((128, CB_W), np.float32)
    wp_flat = Wp.transpose(1, 0, 2).reshape(D, 128)           # [d, (p,o)]
    for k in range(6):
        cb[:, OFF_WP + k * 128:OFF_WP + (k + 1) * 128] = \
            wp_flat[k * 128:(k + 1) * 128, :]
    for p in range(P):
        for i in range(PD):
            cb[p * 16 + i, OFF_SSEL + p] = 1.0
            cb[p, OFF_SBC + p * 16 + i] = 1.0
    cb[:, OFF_M0:OFF_M0 + 80] = 0.2 * W.transpose(0, 2, 1, 3).reshape(128, 80)
    for c in range(C):
        for j in range(CD):
            cb[c * 16 + j, OFF_JSEL + c] = 1.0
            cb[c, OFF_G80 + c * 16 + j] = 1.0
    for c in range(C):
        for p in range(P):
            cb[c * 16 + p, OFF_CSEL + p] = 1.0
            cb[p, OFF_CBCN + c * 16 + p] = -1.0
    for c in range(C):
        # WTa_c rows (c*16+j), cols (p*16+i) = W[p,c,i,j]
        cb[c * 16:(c + 1) * 16, OFF_WTA + c * 128:OFF_WTA + (c + 1) * 128] = \
            W[:, c].transpose(2, 0, 1).reshape(16, 128)
        for p in range(P):
            cb[p * 16:(p + 1) * 16, OFF_ASEL + c * 80 + c * 16 + p] = 1.0
            cb[c * 16 + p, OFF_BSEL + c * 128 + p * 16:
               OFF_BSEL + c * 128 + (p + 1) * 16] = 1.0
        cb[:, OFF_WF + c * 80 + c * 16:OFF_WF + c * 80 + (c + 1) * 16] = \
            W[:, c].reshape(128, 16)

    import ml_dtypes
    cstb = cb.astype(ml_dtypes.bfloat16)
    return np.ascontiguousarray(cstf), np.ascontiguousarray(cstb)


def build_nc(nt: int = NT) -> bass.Bass:
    bc = nt * NB
    nc = bacc.Bacc(None)

    x_d = nc.declare_dram_parameter("xc", [bc, D], F32R, isOutput=False)
    cf_d = nc.declare_dram_parameter("cstf", [128, CF_W], F32R, isOutput=False)
    cb_d = nc.declare_dram_parameter("cstb", [128, CB_W], BF16, isOutput=False)
    v_d = nc.declare_dram_parameter("vout", [nt * 80, NB], BF16, isOutput=True)

    with tile.TileContext(nc) as tc, nc.allow_low_precision(reason="bf16 kernel"):
        with (
            tc.sbuf_pool(name="const", bufs=1) as cpool,
            tc.sbuf_pool(name="xin", bufs=2) as xpool,
            tc.sbuf_pool(name="xt", bufs=3) as xtpool,
            tc.sbuf_pool(name="mid", bufs=3) as mpool,
            tc.sbuf_pool(name="ubank", bufs=nt) as upool,
            tc.sbuf_pool(name="tmul", bufs=4) as tpool,
            tc.sbuf_pool(name="rt", bufs=4) as rtpool,
            tc.sbuf_pool(name="sm", bufs=3) as smpool,
            tc.sbuf_pool(name="out", bufs=3) as opool,
            tc.psum_pool(name="pA", bufs=2) as pA,          # pt/pu/ps/pvt
            tc.psum_pool(name="psm", bufs=1) as psm,        # small f32
            tc.psum_pool(name="pbc", bufs=3) as pbc,        # bcasts
            tc.psum_pool(name="pacc", bufs=2) as pacc,      # logits b
        ):
            # ---- constants: one DMA each, staged through DVE ----
            cf0 = cpool.tile([128, CF_W], F32R)
            nc.sync.dma_start(out=cf0[:], in_=cf_d[:])
            cf = cpool.tile([128, CF_W], F32R)
            nc.vector.tensor_copy(cf[:], cf0[:])
            cb0 = cpool.tile([128, CB_W], BF16)
            nc.sync.dma_start(out=cb0[:], in_=cb_d[:])
            cbs = cpool.tile([128, CB_W], BF16)
            nc.vector.tensor_copy(cbs[:], cb0[:])

            identf = cf[:, 0:128]
            bpf = cf[:, 128:129].bitcast(F32)

            def mm(out, lhsT, rhs, start=True, stop=True):
                nc.tensor.matmul(out, lhsT, rhs, start=start, stop=stop)

            def g_chain(pvq, rows, tag):
                """g = sqrt(s)/(1+s) = exp(0.5*ln(s) - ln(1+s))."""
                l1 = smpool.tile([rows, NB], F32, tag=f"l1{tag}")
                nc.scalar.activation(l1[:], pvq[:], AF.Ln)
                l2 = smpool.tile([rows, NB], F32, tag=f"l2{tag}")
                nc.scalar.activation(l2[:], pvq[:], AF.Ln, bias=1.0)
                gm = smpool.tile([rows, NB], F32, tag=f"gm{tag}")
                nc.vector.scalar_tensor_tensor(
                    gm[:], l1[:], 0.5, l2[:],
                    op0=AluOpType.mult, op1=AluOpType.subtract)
                g = smpool.tile([rows, NB], BF16, tag=f"g{tag}")
                nc.scalar.activation(g[:], gm[:], AF.Exp)
                return g

            u_tiles = []

            # ================= PASS 1: x -> u =================
            for it in range(nt):
                x_sb = xpool.tile([128, 4, D], F32R, tag="xin")
                src = x_d[it * NB:(it + 1) * NB, :].rearrange(
                    "(p q) d -> p q d", p=128)
                nc.sync.dma_start(out=x_sb[:], in_=src)

                xT = xtpool.tile([128, 6, NB], BF16, tag="xt")
                pu = pacc.tile([128, NB], F32, tag="pacc")
                for k in range(6):
                    pt = pA.tile([128, NB], F32R, tag="pA")
                    for q in range(4):
                        nc.tensor.transpose(
                            pt[:, q * 128:(q + 1) * 128],
                            x_sb[:, q, k * 128:(k + 1) * 128],
                            identf)
                    nc.vector.tensor_copy(xT[:, k, :], pt[:])
                    mm(pu[:], cbs[:, OFF_WP + k * 128:OFF_WP + (k + 1) * 128],
                       xT[:, k, :], start=(k == 0), stop=(k == 5))
                u_pre = mpool.tile([128, NB], BF16, tag="upre")
                nc.scalar.activation(u_pre[:], pu[:], AF.Identity, bias=bpf)
                usq = mpool.tile([128, NB], BF16, tag="usq")
                nc.scalar.activation(usq[:], pu[:], AF.Square, bias=bpf)

                psq = psm.tile([8, NB], F32, tag="psm")
                mm(psq[:], cbs[:, OFF_SSEL:OFF_SSEL + 8], usq[:])
                f = g_chain(psq, 8, "f")
                pfb = pbc.tile([128, NB], F32, tag="pbc")
                mm(pfb[:], cbs[:8, OFF_SBC:OFF_SBC + 128], f[:])
                u = upool.tile([128, NB], BF16, tag="u")
                nc.vector.tensor_mul(u[:], u_pre[:], pfb[:])
                u_tiles.append(u)

            # ================= PASS 2: routing (tiles interleaved in pairs
            # so each engine always has an independent instruction ready) ===
            state = {}

            def emit_iter(it, itr):
                u = u_tiles[it]
                st = state.setdefault(it, {})
                pa = st.get("pa")

                ps = pA.tile([80, NB], F32, tag="pA")
                if itr == 0:
                    mm(ps[:], cbs[:, OFF_M0:OFF_M0 + 80], u[:])
                else:
                    e = rtpool.tile([80, NB], BF16, tag="rt_e")
                    nc.scalar.activation(e[:], pa[:], AF.Exp)
                    pden = psm.tile([8, NB], F32, tag="psm")
                    mm(pden[:], cbs[:80, OFF_CSEL:OFF_CSEL + 8], e[:])
                    lse = smpool.tile([8, NB], BF16, tag="lse")
                    nc.scalar.activation(lse[:], pden[:], AF.Ln)
                    mm(pa[:], cbs[:8, OFF_CBCN:OFF_CBCN + 80], lse[:],
                       start=False, stop=True)
                    cn = rtpool.tile([80, NB], BF16, tag="rt_cn")
                    nc.scalar.activation(cn[:], pa[:], AF.Exp)
                    t5 = tpool.tile([128, 5, NB], BF16, tag="t5")
                    # classes 0/2/4 contract only their 8 live rows, at
                    # 32-aligned bases -> concurrent PE row-group tiles
                    for c in (0, 2, 4, 1, 3):
                        pcb = pbc.tile([128, NB], F32, tag="pbc")
                        if c in (0, 2, 4):
                            mm(pcb[:],
                               cbs[c * 16:c * 16 + 8,
                                   OFF_BSEL + c * 128:
                                   OFF_BSEL + (c + 1) * 128],
                               cn[c * 16:c * 16 + 8, :])
                        else:
                            mm(pcb[:],
                               cbs[:80, OFF_BSEL + c * 128:
                                   OFF_BSEL + (c + 1) * 128], cn[:])
                        nc.vector.tensor_mul(t5[:, c, :], u[:], pcb[:])
                    for ci, c in enumerate((0, 2, 4, 1, 3)):
                        mm(ps[:], cbs[:, OFF_WF + c * 80:
                                      OFF_WF + (c + 1) * 80],
                           t5[:, c, :], start=(ci == 0), stop=(ci == 4))

                s_sb = rtpool.tile([80, NB], BF16, tag="rt_s")
                nc.scalar.copy(s_sb[:], ps[:])
                ssq = rtpool.tile([80, NB], BF16, tag="rt_ssq")
                nc.scalar.activation(ssq[:], ps[:], AF.Square)
                pvq = psm.tile([5, NB], F32, tag="psm")
                mm(pvq[:], cbs[:80, OFF_JSEL:OFF_JSEL + 5], ssq[:])
                g = g_chain(pvq, 5, "g")
                pg80 = pbc.tile([80, NB], F32, tag="pbc")
                mm(pg80[:], cbs[:5, OFF_G80:OFF_G80 + 80], g[:])
                v_sb = rtpool.tile([80, NB], BF16, tag="rt_v")
                nc.vector.tensor_mul(v_sb[:], s_sb[:], pg80[:])
                st["v"] = v_sb

                if itr < 2:
                    if itr == 0:
                        pa = pacc.tile([80, NB], F32, tag="pacc")
                        st["pa"] = pa
                    t5a = tpool.tile([128, 5, NB], BF16, tag="t5")
                    for c in (0, 2, 4, 1, 3):
                        pwv = pbc.tile([128, NB], F32, tag="pbc")
                        if c in (0, 2, 4):
                            mm(pwv[:],
                               cbs[c * 16:(c + 1) * 16,
                                   OFF_WTA + c * 128:
                                   OFF_WTA + (c + 1) * 128],
                               v_sb[c * 16:(c + 1) * 16, :])
                        else:
                            mm(pwv[:],
                               cbs[:80, OFF_WTA + c * 128:
                                   OFF_WTA + (c + 1) * 128], v_sb[:])
                        nc.vector.tensor_mul(t5a[:, c, :], u[:], pwv[:])
                    for ci, c in enumerate((0, 2, 4, 1, 3)):
                        mm(pa[:], cbs[:, OFF_ASEL + c * 80:
                                      OFF_ASEL + (c + 1) * 80],
                           t5a[:, c, :],
                           start=(itr == 0 and ci == 0), stop=(ci == 4))

            def emit_out(it):
                v_sb = state[it]["v"]
                nc.sync.dma_start(out=v_d[it * 80:(it + 1) * 80, :],
                                  in_=v_sb[:])

            for tp in range(0, nt, 2):
                pair = [t for t in (tp, tp + 1) if t < nt]
                for itr in range(3):
                    for t in pair:
                        emit_iter(t, itr)
                for t in pair:
                    emit_out(t)

    nc.compile()
    return nc


_NC_CACHE: dict = {}


def _get_nc(nt: int) -> bass.Bass:
    if nt not in _NC_CACHE:
        _NC_CACHE[nt] = build_nc(nt)
    return _NC_CACHE[nt]


def kernel(x, Wp, bp, W):
    x = np.asarray(x, np.float32)
    cstf, cstb = build_consts(Wp, bp, W)
    nc = _get_nc(NT)
    in_maps = [{"xc": np.ascontiguousarray(x[i * BC:(i + 1) * BC]),
                "cstf": cstf, "cstb": cstb}
               for i in range(NCORES)]
    res = run_bass_kernel_spmd(nc, in_maps, list(range(NCORES)))
    parts = []
    for i in range(NCORES):
        arr = np.asarray(res.results[i]["vout"]).astype(np.float32)
        # [nt*80, 512] -> [nt, 80, 4(q), 128(p)] -> rows it*512 + p*4 + q
        arr = arr.reshape(NT, 80, 4, 128).transpose(0, 3, 2, 1).reshape(BC, 80)
        parts.append(arr)
    return np.concatenate(parts, axis=0).reshape(B, C, CD)
